# Initial kernel scaffold
#
"""Trainium2 Bass kernel: pairwise cosine similarity (retrieval_knn).

out[i, j] = <img_i, txt_j> / max(||img_i|| * ||txt_j||, 1e-8)

Strategy (data-parallel over 8 NeuronCores):
  - shard img_feature rows 8-ways: each core computes a [1024, 8192] slab
  - text_feature replicated (transposed on host to d-major for the PE)
  - on-device: norms of both operands, hi/lo fp32r-split matmul (fp32-grade
    accuracy at 1 cycle/row/term), epilogue scaling by 1/(|img| |txt|)

Self-contained: hardcodes shapes; only imports the runtime from /opt/trn_rl_repo.
"""
import sys

if '/opt/trn_rl_repo' not in sys.path:
    sys.path.insert(0, '/opt/trn_rl_repo')

import numpy as np

B = 8192          # rows of img_feature and text_feature
D = 512           # feature dim (contraction)
NCORES = 8
BI = B // NCORES  # img rows per core = 1024
JB = 2048         # j-block streamed per iteration
NJB = B // JB     # 4
NSUB = 512        # matmul moving-dim (max for fp32 PSUM bank)
NDC = D // 128    # 4 contraction chunks of 128

# "split": decompose fp32 into hi+lo fp32r parts; 3 matmuls per tile pair
#          (hi@hi + hi@lo + lo@hi) -> ~fp32 accuracy at 3 cyc/row.
# "f32r":  single fp32r matmul -> ~12-bit mantissa, 1 cyc/row, memory-bound.
MODE = "split"

_cache = {}


def _build(mode):
    import concourse.bacc as bacc
    import concourse.mybir as mybir
    from concourse.tile import TileContext

    f32 = mybir.dt.float32
    f32r = mybir.dt.float32r
    Act = mybir.ActivationFunctionType
    Alu = mybir.AluOpType

    nc = bacc.Bacc()

    a_t_d = nc.dram_tensor("a_t", [D, BI], f32, kind="ExternalInput")
    a_nat_d = nc.dram_tensor("a_nat", [BI, D], f32, kind="ExternalInput")
    t_t_d = nc.dram_tensor("t_t", [D, B], f32, kind="ExternalInput")
    out_d = nc.dram_tensor("out", [BI, B], f32, kind="ExternalOutput")

    split = mode == "split"

    with TileContext(nc) as tc:
        with tc.tile_pool(name="res", bufs=1) as res, \
             tc.tile_pool(name="anat", bufs=2) as anat_p, \
             tc.tile_pool(name="atld", bufs=2) as atld_p, \
             tc.tile_pool(name="tt", bufs=3) as tt_p, \
             tc.tile_pool(name="sqr", bufs=2) as sqr_p, \
             tc.tile_pool(name="rtbc", bufs=2) as rtbc_p, \
             tc.tile_pool(name="outs", bufs=3) as outs_p, \
             tc.tile_pool(name="pout", bufs=4, space="PSUM") as pout_p, \
             tc.tile_pool(name="pnt", bufs=4, space="PSUM") as pnt_p:

            # ---------------- phase A: img-side prep ----------------
            ssA = res.tile([128, BI // 128], f32, name="ssA")
            sq_scr = res.tile([128, D], f32, name="sq_scr")
            for ic in range(BI // 128):
                a_nat_t = anat_p.tile([128, D], f32, name=f"an{ic}", tag="an")
                nc.sync.dma_start(out=a_nat_t[:],
                                  in_=a_nat_d[ic * 128:(ic + 1) * 128, :])
                nc.scalar.activation(sq_scr[:], a_nat_t[:], Act.Square,
                                     accum_out=ssA[:, ic:ic + 1])
            nrmA = res.tile([128, BI // 128], f32, name="nrmA")
            nc.scalar.sqrt(nrmA[:], ssA[:])
            rA = res.tile([128, BI // 128], f32, name="rA")
            nc.vector.reciprocal(rA[:], nrmA[:])

            Ar = [res.tile([128, BI], f32r, name=f"ar{dc}") for dc in range(NDC)]
            Al = [res.tile([128, BI], f32r, name=f"al{dc}") for dc in range(NDC)] \
                if split else None
            for dc in range(NDC):
                a_t_t = atld_p.tile([128, BI], f32, name=f"atl{dc}", tag="atl")
                nc.sync.dma_start(out=a_t_t[:],
                                  in_=a_t_d[dc * 128:(dc + 1) * 128, :])
                nc.vector.tensor_copy(Ar[dc][:], a_t_t[:])
                if split:
                    nc.vector.tensor_sub(Al[dc][:], a_t_t[:],
                                         Ar[dc][:].bitcast(f32))

            ones_col = res.tile([128, 1], f32r, name="ones_col")
            nc.vector.memset(ones_col[:], 1.0)

            # ---------------- phase B: stream T, matmul, scale ----------------
            for jb in range(NJB):
                tt = []
                for dc in range(NDC):
                    t = tt_p.tile([128, JB], f32, name=f"tt{jb}_{dc}", tag="tt")
                    nc.sync.dma_start(
                        out=t[:],
                        in_=t_t_d[dc * 128:(dc + 1) * 128, jb * JB:(jb + 1) * JB])
                    tt.append(t)

                # text norms: sum_d t^2 via ones-matmul on squared tiles
                pnt = [pnt_p.tile([1, NSUB], f32, name=f"nt{jb}_{js}", tag="nt")
                       for js in range(JB // NSUB)]
                for dc in range(NDC):
                    sq = sqr_p.tile([128, JB], f32r, name=f"sq{jb}_{dc}", tag="sq")
                    nc.vector.tensor_mul(sq[:], tt[dc][:], tt[dc][:])
                    for js in range(JB // NSUB):
                        nc.tensor.matmul(
                            pnt[js][:],
                            ones_col[:],
                            sq[:, js * NSUB:(js + 1) * NSUB],
                            start=(dc == 0), stop=(dc == NDC - 1))

                nt_row = res.tile([1, JB], f32, name=f"ntr{jb}", tag="ntr",
                                  bufs=2)
                for js in range(JB // NSUB):
                    nc.scalar.sqrt(nt_row[:, js * NSUB:(js + 1) * NSUB],
                                   pnt[js][:])
                rT_row = res.tile([1, JB], f32, name=f"rtr{jb}", tag="rtr",
                                  bufs=2)
                nc.vector.reciprocal(rT_row[:], nt_row[:])
                rT_bc = rtbc_p.tile([128, JB], f32, name=f"rtb{jb}", tag="rtb")
                nc.gpsimd.partition_broadcast(rT_bc[:], rT_row[:])

                # decompose T tiles (unscaled; rT applied in epilogue)
                Br, Bl = [], []
                for dc in range(NDC):
                    br = res.tile([128, JB], f32r, name=f"br{jb}_{dc}",
                                  tag=f"br{dc}", bufs=1)
                    nc.vector.tensor_copy(br[:], tt[dc][:])
                    Br.append(br)
                    if split:
                        bl = res.tile([128, JB], f32r, name=f"bl{jb}_{dc}",
                                      tag=f"bl{dc}", bufs=1)
                        nc.vector.tensor_sub(bl[:], tt[dc][:],
                                             br[:].bitcast(f32))
                        Bl.append(bl)

                for ic in range(BI // 128):
                    pouts = [pout_p.tile([128, NSUB], f32,
                                         name=f"po{jb}_{ic}_{js}", tag="po")
                             for js in range(JB // NSUB)]
                    n_terms = 3 if split else 1
                    for dc in range(NDC):
                        if split:
                            pairs = ((Ar[dc], Br[dc]), (Ar[dc], Bl[dc]),
                                     (Al[dc], Br[dc]))
                        else:
                            pairs = ((Ar[dc], Br[dc]),)
                        for ti, (lt_, rt_) in enumerate(pairs):
                            for js in range(JB // NSUB):
                                nc.tensor.matmul(
                                    pouts[js][:],
                                    lt_[:, ic * 128:(ic + 1) * 128],
                                    rt_[:, js * NSUB:(js + 1) * NSUB],
                                    start=(dc == 0 and ti == 0),
                                    stop=(dc == NDC - 1 and ti == n_terms - 1))

                    outs = outs_p.tile([128, JB], f32, name=f"os{jb}_{ic}",
                                       tag="os")
                    for js in range(JB // NSUB):
                        nc.scalar.activation(
                            outs[:, js * NSUB:(js + 1) * NSUB], pouts[js][:],
                            Act.Copy, scale=rA[:, ic:ic + 1])
                    nc.vector.tensor_mul(outs[:], outs[:], rT_bc[:])
                    nc.sync.dma_start(
                        out=out_d[ic * 128:(ic + 1) * 128,
                                  jb * JB:(jb + 1) * JB],
                        in_=outs[:])

    nc.compile()
    return nc


def kernel(img_feature, text_feature, text_lens=None, **_):
    from concourse.bass_utils import run_bass_kernel_spmd

    if MODE not in _cache:
        _cache[MODE] = _build(MODE)
    nc = _cache[MODE]

    img = np.ascontiguousarray(np.asarray(img_feature, dtype=np.float32))
    txt = np.ascontiguousarray(np.asarray(text_feature, dtype=np.float32))
    a_t_full = np.ascontiguousarray(img.T)          # [D, B]
    t_t = np.ascontiguousarray(txt.T)               # [D, B]

    in_maps = []
    for c in range(NCORES):
        in_maps.append({
            "a_t": np.ascontiguousarray(a_t_full[:, c * BI:(c + 1) * BI]),
            "a_nat": np.ascontiguousarray(img[c * BI:(c + 1) * BI, :]),
            "t_t": t_t,
        })

    res = run_bass_kernel_spmd(nc, in_maps, list(range(NCORES))).results
    return np.concatenate([res[c]["out"] for c in range(NCORES)], axis=0)


# revision 7
# speedup vs baseline: 1.8376x; 1.8376x over previous
"""Trainium2 Bass kernel: pairwise cosine similarity (retrieval_knn).

out[i, j] = <img_i, txt_j> / max(||img_i|| * ||txt_j||, 1e-8)

Strategy (data-parallel over 8 NeuronCores):
  - shard img_feature rows 8-ways: each core computes a [1024, 8192] slab
  - text_feature replicated (transposed on host to d-major for the PE)
  - on-device: norms of both operands, hi/lo fp32r-split matmul (fp32-grade
    accuracy at 1 cycle/row/term), epilogue scaling by 1/(|img| |txt|)

Self-contained: hardcodes shapes; only imports the runtime from /opt/trn_rl_repo.
"""
import sys

if '/opt/trn_rl_repo' not in sys.path:
    sys.path.insert(0, '/opt/trn_rl_repo')

import numpy as np

B = 8192          # rows of img_feature and text_feature
D = 512           # feature dim (contraction)
NCORES = 8
BI = B // NCORES  # img rows per core = 1024
JB = 1024         # j-block streamed per iteration
NJB = B // JB     # 4
NSUB = 512        # matmul moving-dim (max for fp32 PSUM bank)
NDC = D // 128    # 4 contraction chunks of 128

# "split": decompose fp32 into hi+lo fp32r parts; 3 matmuls per tile pair
#          (hi@hi + hi@lo + lo@hi) -> ~fp32 accuracy at 3 cyc/row.
# "f32r":  single fp32r matmul -> ~12-bit mantissa, 1 cyc/row, memory-bound.
MODE = "f32r"

_cache = {}


def _build(mode):
    import concourse.bacc as bacc
    import concourse.mybir as mybir
    from concourse.tile import TileContext

    f32 = mybir.dt.float32
    f32r = mybir.dt.float32r
    Act = mybir.ActivationFunctionType
    Alu = mybir.AluOpType

    nc = bacc.Bacc()

    a_t_d = nc.dram_tensor("a_t", [D, BI], f32, kind="ExternalInput")
    a_nat_d = nc.dram_tensor("a_nat", [BI, D], f32, kind="ExternalInput")
    t_t_d = nc.dram_tensor("t_t", [D, B], f32, kind="ExternalInput")
    out_d = nc.dram_tensor("out", [BI, B], f32, kind="ExternalOutput")

    split = mode == "split"

    with TileContext(nc) as tc:
        with tc.tile_pool(name="res", bufs=1) as res, \
             tc.tile_pool(name="anat", bufs=2) as anat_p, \
             tc.tile_pool(name="atld", bufs=2) as atld_p, \
             tc.tile_pool(name="tt", bufs=8) as tt_p, \
             tc.tile_pool(name="sqr", bufs=2) as sqr_p, \
             tc.tile_pool(name="tsc", bufs=2) as tsc_p, \
             tc.tile_pool(name="rtbc", bufs=2) as rtbc_p, \
             tc.tile_pool(name="outs", bufs=4) as outs_p, \
             tc.tile_pool(name="pout", bufs=2, space="PSUM") as pout_p, \
             tc.tile_pool(name="pnt", bufs=2, space="PSUM") as pnt_p:

            # ---------------- phase A: img-side prep ----------------
            # exact fp32 norms of img rows from the natural layout
            ssA = res.tile([128, BI // 128], f32, name="ssA")
            sq_scr = res.tile([128, D], f32, name="sq_scr")
            for ic in range(BI // 128):
                a_nat_t = anat_p.tile([128, D], f32, name=f"an{ic}", tag="an")
                nc.sync.dma_start(out=a_nat_t[:],
                                  in_=a_nat_d[ic * 128:(ic + 1) * 128, :])
                nc.scalar.activation(sq_scr[:], a_nat_t[:], Act.Square,
                                     accum_out=ssA[:, ic:ic + 1])
            nrmA = res.tile([128, BI // 128], f32, name="nrmA")
            nc.scalar.sqrt(nrmA[:], ssA[:])
            rA = res.tile([128, BI // 128], f32, name="rA")
            nc.vector.reciprocal(rA[:], nrmA[:])

            Ar = [res.tile([128, BI], f32r, name=f"ar{dc}") for dc in range(NDC)]
            Al = [res.tile([128, BI], f32r, name=f"al{dc}") for dc in range(NDC)] \
                if split else None
            for dc in range(NDC):
                a_t_t = atld_p.tile([128, BI], f32, name=f"atl{dc}", tag="atl")
                nc.sync.dma_start(out=a_t_t[:],
                                  in_=a_t_d[dc * 128:(dc + 1) * 128, :])
                nc.vector.tensor_copy(Ar[dc][:], a_t_t[:])
                if split:
                    nc.vector.tensor_sub(Al[dc][:], a_t_t[:],
                                         Ar[dc][:].bitcast(f32))

            ones_f32 = res.tile([128, 1], f32, name="ones_f32")
            nc.vector.memset(ones_f32[:], 1.0)
            ones_col = res.tile([128, 1], f32r, name="ones_col")
            nc.vector.tensor_copy(ones_col[:], ones_f32[:])

            # ---------------- phase B: stream T, matmul, scale ----------------
            # norm_path(jb) is emitted one iteration ahead of main_mms(jb)
            # so the sqrt->recip->broadcast->scale chain for jb+1 hides
            # under jb's main matmul burst (keeps the PE dense / HAM warm).
            def norm_path(jb):
                tt = []
                for dc in range(NDC):
                    t = tt_p.tile([128, JB], f32, name=f"tt{jb}_{dc}", tag="tt")
                    nc.sync.dma_start(
                        out=t[:],
                        in_=t_t_d[dc * 128:(dc + 1) * 128, jb * JB:(jb + 1) * JB])
                    tt.append(t)

                # text norms: sum_d t^2 via ones-matmul on squared tiles
                pnt = pnt_p.tile([1, JB], f32, name=f"nt{jb}", tag="nt")
                for dc in range(NDC):
                    sq = sqr_p.tile([128, JB], f32r, name=f"sq{jb}_{dc}", tag="sq")
                    nc.vector.tensor_mul(sq[:], tt[dc][:], tt[dc][:])
                    for js in range(JB // NSUB):
                        nc.tensor.matmul(
                            pnt[:, js * NSUB:(js + 1) * NSUB],
                            ones_col[:],
                            sq[:, js * NSUB:(js + 1) * NSUB],
                            start=(dc == 0), stop=(dc == NDC - 1))

                nt_row = res.tile([1, JB], f32, name=f"ntr{jb}", tag="ntr",
                                  bufs=2)
                nc.scalar.sqrt(nt_row[:], pnt[:])
                rT_row = res.tile([1, JB], f32, name=f"rtr{jb}", tag="rtr",
                                  bufs=2)
                nc.vector.reciprocal(rT_row[:], nt_row[:])
                rT_bc = rtbc_p.tile([128, JB], f32, name=f"rtb{jb}", tag="rtb")
                nc.gpsimd.partition_broadcast(rT_bc[:], rT_row[:])

                # scale T columns by 1/|txt_j| and round to f32r (fused)
                Br, Bl = [], []
                for dc in range(NDC):
                    br = res.tile([128, JB], f32r, name=f"br{jb}_{dc}",
                                  tag=f"br{dc}", bufs=2)
                    if split:
                        tsc = tsc_p.tile([128, JB], f32, name=f"ts{jb}_{dc}",
                                         tag="ts")
                        nc.vector.tensor_mul(tsc[:], tt[dc][:], rT_bc[:])
                        nc.vector.tensor_copy(br[:], tsc[:])
                        bl = res.tile([128, JB], f32r, name=f"bl{jb}_{dc}",
                                      tag=f"bl{dc}", bufs=2)
                        nc.vector.tensor_sub(bl[:], tsc[:], br[:].bitcast(f32))
                        Bl.append(bl)
                    else:
                        nc.vector.tensor_mul(br[:], tt[dc][:], rT_bc[:])
                    Br.append(br)
                return Br, Bl

            staged = norm_path(0)
            for jb in range(NJB):
                Br, Bl = staged
                if jb + 1 < NJB:
                    staged = norm_path(jb + 1)

                for ic in range(BI // 128):
                    pout = pout_p.tile([128, JB], f32, name=f"po{jb}_{ic}",
                                       tag="po")
                    n_terms = 3 if split else 1
                    for js in range(JB // NSUB):
                        for dc in range(NDC):
                            if split:
                                pairs = ((Ar[dc], Br[dc]), (Ar[dc], Bl[dc]),
                                         (Al[dc], Br[dc]))
                            else:
                                pairs = ((Ar[dc], Br[dc]),)
                            for ti, (lt_, rt_) in enumerate(pairs):
                                nc.tensor.matmul(
                                    pout[:, js * NSUB:(js + 1) * NSUB],
                                    lt_[:, ic * 128:(ic + 1) * 128],
                                    rt_[:, js * NSUB:(js + 1) * NSUB],
                                    start=(dc == 0 and ti == 0),
                                    stop=(dc == NDC - 1 and ti == n_terms - 1))

                    outs = outs_p.tile([128, JB], f32, name=f"os{jb}_{ic}",
                                       tag="os")
                    nc.scalar.activation(outs[:], pout[:], Act.Copy,
                                         scale=rA[:, ic:ic + 1])
                    nc.sync.dma_start(
                        out=out_d[ic * 128:(ic + 1) * 128,
                                  jb * JB:(jb + 1) * JB],
                        in_=outs[:])

    nc.compile()
    return nc


def kernel(img_feature, text_feature, text_lens=None, **_):
    from concourse.bass_utils import run_bass_kernel_spmd

    if MODE not in _cache:
        _cache[MODE] = _build(MODE)
    nc = _cache[MODE]

    img = np.ascontiguousarray(np.asarray(img_feature, dtype=np.float32))
    txt = np.ascontiguousarray(np.asarray(text_feature, dtype=np.float32))
    a_t_full = np.ascontiguousarray(img.T)          # [D, B]
    t_t = np.ascontiguousarray(txt.T)               # [D, B]

    in_maps = []
    for c in range(NCORES):
        in_maps.append({
            "a_t": np.ascontiguousarray(a_t_full[:, c * BI:(c + 1) * BI]),
            "a_nat": np.ascontiguousarray(img[c * BI:(c + 1) * BI, :]),
            "t_t": t_t,
        })

    res = run_bass_kernel_spmd(nc, in_maps, list(range(NCORES))).results
    return np.concatenate([res[c]["out"] for c in range(NCORES)], axis=0)


# revision 10
# speedup vs baseline: 2.1201x; 1.1537x over previous
"""Trainium2 Bass kernel: pairwise cosine similarity (retrieval_knn).

out[i, j] = <img_i, txt_j> / max(||img_i|| * ||txt_j||, 1e-8)

Strategy (data-parallel over 8 NeuronCores):
  - shard img_feature rows 8-ways: each core computes a [1024, 8192] slab
  - text_feature replicated (transposed on host to d-major for the PE)
  - on-device: norms of both operands, hi/lo fp32r-split matmul (fp32-grade
    accuracy at 1 cycle/row/term), epilogue scaling by 1/(|img| |txt|)

Self-contained: hardcodes shapes; only imports the runtime from /opt/trn_rl_repo.
"""
import sys

if '/opt/trn_rl_repo' not in sys.path:
    sys.path.insert(0, '/opt/trn_rl_repo')

import numpy as np

B = 8192          # rows of img_feature and text_feature
D = 512           # feature dim (contraction)
NCORES = 8
BI = B // NCORES  # img rows per core = 1024
JB = 1024         # j-block streamed per iteration
NJB = B // JB     # 4
NSUB = 512        # matmul moving-dim (max for fp32 PSUM bank)
NDC = D // 128    # 4 contraction chunks of 128

# "split": decompose fp32 into hi+lo fp32r parts; 3 matmuls per tile pair
#          (hi@hi + hi@lo + lo@hi) -> ~fp32 accuracy at 3 cyc/row.
# "f32r":  single fp32r matmul -> ~12-bit mantissa, 1 cyc/row, memory-bound.
MODE = "f32r"

_cache = {}


def _build(mode):
    import concourse.bacc as bacc
    import concourse.mybir as mybir
    from concourse.tile import TileContext

    f32 = mybir.dt.float32
    f32r = mybir.dt.float32r
    Act = mybir.ActivationFunctionType
    Alu = mybir.AluOpType

    nc = bacc.Bacc()

    a_t_d = nc.dram_tensor("a_t", [D, BI], f32, kind="ExternalInput")
    a_nat_d = nc.dram_tensor("a_nat", [BI, D], f32, kind="ExternalInput")
    t_t_d = nc.dram_tensor("t_t", [D, B], f32, kind="ExternalInput")
    out_d = nc.dram_tensor("out", [BI, B], f32, kind="ExternalOutput")

    split = mode == "split"

    with TileContext(nc) as tc:
        with tc.tile_pool(name="res", bufs=1) as res, \
             tc.tile_pool(name="anat", bufs=2) as anat_p, \
             tc.tile_pool(name="atld", bufs=2) as atld_p, \
             tc.tile_pool(name="tt", bufs=8) as tt_p, \
             tc.tile_pool(name="sqr", bufs=2) as sqr_p, \
             tc.tile_pool(name="tsc", bufs=2) as tsc_p, \
             tc.tile_pool(name="rtbc", bufs=2) as rtbc_p, \
             tc.tile_pool(name="outs", bufs=4) as outs_p, \
             tc.tile_pool(name="pout", bufs=2, space="PSUM") as pout_p, \
             tc.tile_pool(name="pnt", bufs=2, space="PSUM") as pnt_p:

            # ---------------- phase A: img-side prep ----------------
            # a_t first: the main matmuls need Ar/Al as early as possible
            Ar = [res.tile([128, BI], f32r, name=f"ar{dc}") for dc in range(NDC)]
            Al = [res.tile([128, BI], f32r, name=f"al{dc}") for dc in range(NDC)] \
                if split else None
            for dc in range(NDC):
                a_t_t = atld_p.tile([128, BI], f32, name=f"atl{dc}", tag="atl")
                nc.sync.dma_start(out=a_t_t[:],
                                  in_=a_t_d[dc * 128:(dc + 1) * 128, :])
                nc.vector.tensor_copy(Ar[dc][:], a_t_t[:])
                if split:
                    nc.vector.tensor_sub(Al[dc][:], a_t_t[:],
                                         Ar[dc][:].bitcast(f32))

            ones_f32 = res.tile([128, 1], f32, name="ones_f32")
            nc.vector.memset(ones_f32[:], 1.0)
            ones_col = res.tile([128, 1], f32r, name="ones_col")
            nc.vector.tensor_copy(ones_col[:], ones_f32[:])

            # exact fp32 norms of img rows from the natural layout
            # (emitted after phase B's first prefetch fires; see below)
            def a_norm_path():
                ssA = res.tile([128, BI // 128], f32, name="ssA")
                sq_scr = res.tile([128, D], f32, name="sq_scr")
                for ic in range(BI // 128):
                    a_nat_t = anat_p.tile([128, D], f32, name=f"an{ic}",
                                          tag="an")
                    nc.sync.dma_start(out=a_nat_t[:],
                                      in_=a_nat_d[ic * 128:(ic + 1) * 128, :])
                    nc.scalar.activation(sq_scr[:], a_nat_t[:], Act.Square,
                                         accum_out=ssA[:, ic:ic + 1])
                nrmA = res.tile([128, BI // 128], f32, name="nrmA")
                nc.scalar.sqrt(nrmA[:], ssA[:])
                rA = res.tile([128, BI // 128], f32, name="rA")
                nc.vector.reciprocal(rA[:], nrmA[:])
                return rA

            # ---------------- phase B: stream T, matmul, scale ----------------
            # norm_path(jb) is emitted one iteration ahead of main_mms(jb)
            # so the sqrt->recip->broadcast->scale chain for jb+1 hides
            # under jb's main matmul burst (keeps the PE dense / HAM warm).
            def norm_path(jb):
                tt = []
                for dc in range(NDC):
                    t = tt_p.tile([128, JB], f32, name=f"tt{jb}_{dc}", tag="tt")
                    nc.sync.dma_start(
                        out=t[:],
                        in_=t_t_d[dc * 128:(dc + 1) * 128, jb * JB:(jb + 1) * JB])
                    tt.append(t)

                # text norms: sum_d t^2 via ones-matmul on squared tiles
                pnt = pnt_p.tile([1, JB], f32, name=f"nt{jb}", tag="nt")
                for dc in range(NDC):
                    sq = sqr_p.tile([128, JB], f32r, name=f"sq{jb}_{dc}", tag="sq")
                    nc.vector.tensor_mul(sq[:], tt[dc][:], tt[dc][:])
                    for js in range(JB // NSUB):
                        nc.tensor.matmul(
                            pnt[:, js * NSUB:(js + 1) * NSUB],
                            ones_col[:],
                            sq[:, js * NSUB:(js + 1) * NSUB],
                            start=(dc == 0), stop=(dc == NDC - 1))

                nt_row = res.tile([1, JB], f32, name=f"ntr{jb}", tag="ntr",
                                  bufs=2)
                nc.scalar.sqrt(nt_row[:], pnt[:])
                rT_row = res.tile([1, JB], f32, name=f"rtr{jb}", tag="rtr",
                                  bufs=2)
                r_scr = res.tile([1, JB], f32, name=f"rsc{jb}", tag="rsc",
                                 bufs=2)
                nc.vector.reciprocal_approx_accurate(rT_row[:], nt_row[:],
                                                     scratch=r_scr[:])
                rT_bc = rtbc_p.tile([128, JB], f32, name=f"rtb{jb}", tag="rtb")
                nc.gpsimd.partition_broadcast(rT_bc[:], rT_row[:])

                # scale T columns by 1/|txt_j| and round to f32r (fused)
                Br, Bl = [], []
                for dc in range(NDC):
                    br = res.tile([128, JB], f32r, name=f"br{jb}_{dc}",
                                  tag=f"br{dc}", bufs=2)
                    if split:
                        tsc = tsc_p.tile([128, JB], f32, name=f"ts{jb}_{dc}",
                                         tag="ts")
                        nc.vector.tensor_mul(tsc[:], tt[dc][:], rT_bc[:])
                        nc.vector.tensor_copy(br[:], tsc[:])
                        bl = res.tile([128, JB], f32r, name=f"bl{jb}_{dc}",
                                      tag=f"bl{dc}", bufs=2)
                        nc.vector.tensor_sub(bl[:], tsc[:], br[:].bitcast(f32))
                        Bl.append(bl)
                    else:
                        nc.vector.tensor_mul(br[:], tt[dc][:], rT_bc[:])
                    Br.append(br)
                return Br, Bl

            staged = norm_path(0)
            rA = a_norm_path()
            for jb in range(NJB):
                Br, Bl = staged
                if jb + 1 < NJB:
                    staged = norm_path(jb + 1)

                for ic in range(BI // 128):
                    pout = pout_p.tile([128, JB], f32, name=f"po{jb}_{ic}",
                                       tag="po")
                    n_terms = 3 if split else 1
                    for js in range(JB // NSUB):
                        for dc in range(NDC):
                            if split:
                                pairs = ((Ar[dc], Br[dc]), (Ar[dc], Bl[dc]),
                                         (Al[dc], Br[dc]))
                            else:
                                pairs = ((Ar[dc], Br[dc]),)
                            for ti, (lt_, rt_) in enumerate(pairs):
                                nc.tensor.matmul(
                                    pout[:, js * NSUB:(js + 1) * NSUB],
                                    lt_[:, ic * 128:(ic + 1) * 128],
                                    rt_[:, js * NSUB:(js + 1) * NSUB],
                                    start=(dc == 0 and ti == 0),
                                    stop=(dc == NDC - 1 and ti == n_terms - 1))

                    outs = outs_p.tile([128, JB], f32, name=f"os{jb}_{ic}",
                                       tag="os")
                    nc.scalar.activation(outs[:], pout[:], Act.Copy,
                                         scale=rA[:, ic:ic + 1])
                    nc.sync.dma_start(
                        out=out_d[ic * 128:(ic + 1) * 128,
                                  jb * JB:(jb + 1) * JB],
                        in_=outs[:])

    nc.compile()
    return nc


def kernel(img_feature, text_feature, text_lens=None, **_):
    from concourse.bass_utils import run_bass_kernel_spmd

    if MODE not in _cache:
        _cache[MODE] = _build(MODE)
    nc = _cache[MODE]

    img = np.ascontiguousarray(np.asarray(img_feature, dtype=np.float32))
    txt = np.ascontiguousarray(np.asarray(text_feature, dtype=np.float32))
    a_t_full = np.ascontiguousarray(img.T)          # [D, B]
    t_t = np.ascontiguousarray(txt.T)               # [D, B]

    in_maps = []
    for c in range(NCORES):
        in_maps.append({
            "a_t": np.ascontiguousarray(a_t_full[:, c * BI:(c + 1) * BI]),
            "a_nat": np.ascontiguousarray(img[c * BI:(c + 1) * BI, :]),
            "t_t": t_t,
        })

    res = run_bass_kernel_spmd(nc, in_maps, list(range(NCORES))).results
    return np.concatenate([res[c]["out"] for c in range(NCORES)], axis=0)
